# revision 1
# baseline (speedup 1.0000x reference)
import sys, time

sys.path.insert(0, "/opt/trn_rl_repo")
import numpy as np
import ml_dtypes
from concourse import bass, bacc, tile, mybir
from concourse.bass_utils import run_bass_kernel_spmd

F32 = mybir.dt.float32
F32R = mybir.dt.float32r
BF16 = mybir.dt.bfloat16

B, N, DIM = 4, 2048, 1024
HEADS, DH = 16, 64
G = 8          # heads per core
GI = G * DH    # 512 = inner width per core
SCALE = DH ** -0.5
NB = N // 128  # 16 j-blocks
NCH = N // 512  # 4 q-chunks

# attention compute dtype for P/V side (bf16 keeps SBUF small; S^T contraction is f32r)
PV_DT = BF16

_CACHE = {}


def _build():
    nc = bacc.Bacc(None, target_bir_lowering=False)
    xT = nc.declare_dram_parameter("xT", [DIM, N], BF16, isOutput=False)
    wq = nc.declare_dram_parameter("wq", [DIM, GI], BF16, isOutput=False)
    wk = nc.declare_dram_parameter("wk", [DIM, GI], BF16, isOutput=False)
    wv = nc.declare_dram_parameter("wv", [DIM, GI], BF16, isOutput=False)
    wo = nc.declare_dram_parameter("wo", [GI, DIM], BF16, isOutput=False)
    msk = nc.declare_dram_parameter("msk", [128, 896], PV_DT, isOutput=False)
    on1 = nc.declare_dram_parameter("on1", [128, 64], BF16, isOutput=False)
    onv = nc.declare_dram_parameter("onv", [128, G], PV_DT, isOutput=False)
    out = nc.declare_dram_parameter("out", [N, DIM], F32, isOutput=True)

    DT = DIM // 128  # 8 dim tiles
    IT = GI // 128   # 4 inner tiles

    with tile.TileContext(nc) as tc:
        with (
            nc.allow_low_precision(reason="attention P/V in bf16, f32r rounding; rel-err gate 2e-2"),
            tc.tile_pool(name="big", bufs=1) as big,
            tc.tile_pool(name="pt", bufs=3) as ptp,
            tc.tile_pool(name="st", bufs=2) as stp,
        ):
            # ---- persistent SBUF ----
            qT = [big.tile([128, N], BF16, name=f"qT{i}", tag=f"qT{i}") for i in range(IT)]
            kT = [big.tile([128, N], BF16, name=f"kT{i}", tag=f"kT{i}") for i in range(IT)]
            # v interleaved with ones col per head: [128, 8*65]
            vg = [big.tile([128, G * (DH + 1)], PV_DT, name=f"v{r}", tag=f"v{r}") for r in range(NB)]
            mask = big.tile([128, 896], PV_DT, tag="mask")
            ones1 = big.tile([128, 64], BF16, tag="ones1")

            p1cm = tc.tile_pool(name="p1", bufs=1)
            p1 = p1cm.__enter__()
            psAcm = tc.tile_pool(name="psA", bufs=3, space="PSUM")
            psA = psAcm.__enter__()
            xTt = [p1.tile([128, N], BF16, name=f"xT{d}", tag=f"xT{d}") for d in range(DT)]
            wqt = [p1.tile([128, GI], BF16, name=f"wq{d}", tag=f"wq{d}") for d in range(DT)]
            wkt = [p1.tile([128, GI], BF16, name=f"wk{d}", tag=f"wk{d}") for d in range(DT)]
            wvt = [p1.tile([128, GI], BF16, name=f"wv{d}", tag=f"wv{d}") for d in range(DT)]

            for d in range(DT):
                nc.sync.dma_start(xTt[d][:], xT[d * 128:(d + 1) * 128, :])
                nc.sync.dma_start(wqt[d][:], wq[d * 128:(d + 1) * 128, :])
                nc.sync.dma_start(wkt[d][:], wk[d * 128:(d + 1) * 128, :])
                nc.sync.dma_start(wvt[d][:], wv[d * 128:(d + 1) * 128, :])
            nc.sync.dma_start(mask[:], msk[:])
            nc.sync.dma_start(ones1[:], on1[:])
            # ones columns of vg via DMA broadcast from onv
            for r in range(NB):
                dst = vg[r][:].rearrange("p (h c) -> p h c", c=DH + 1)[:, :, DH:DH + 1]
                nc.sync.dma_start(dst, onv[:].rearrange("p (h c) -> p h c", c=1))

            # ---- projections ----
            # qT/kT: [inner, rows] = w.T @ xT ; lhsT = w block, rhs = xT block
            for it in range(IT):
                for rc in range(N // 512):
                    pq = psA.tile([128, 512], F32, name="pq", tag="pproj")
                    pk = psA.tile([128, 512], F32, name="pk", tag="pproj")
                    for d in range(DT):
                        nc.tensor.matmul(
                            pq[:], wqt[d][:, it * 128:(it + 1) * 128],
                            xTt[d][:, rc * 512:(rc + 1) * 512],
                            start=(d == 0), stop=(d == DT - 1))
                    for d in range(DT):
                        nc.tensor.matmul(
                            pk[:], wkt[d][:, it * 128:(it + 1) * 128],
                            xTt[d][:, rc * 512:(rc + 1) * 512],
                            start=(d == 0), stop=(d == DT - 1))
                    nc.vector.tensor_copy(qT[it][:, rc * 512:(rc + 1) * 512], pq[:])
                    nc.vector.tensor_copy(kT[it][:, rc * 512:(rc + 1) * 512], pk[:])
            # v: [rows, inner] ; lhsT = xT block, rhs = w_v block -> strided into vg
            for r in range(NB):
                pv = psA.tile([128, 512], F32, name="pv", tag="pproj")
                for d in range(DT):
                    nc.tensor.matmul(
                        pv[:], xTt[d][:, r * 128:(r + 1) * 128], wvt[d][:],
                        start=(d == 0), stop=(d == DT - 1))
                dst = vg[r][:].rearrange("p (h c) -> p h c", c=DH + 1)[:, :, 0:DH]
                nc.vector.tensor_copy(dst, pv[:].rearrange("p (h c) -> p h c", c=DH))

            p1cm.__exit__(None, None, None)
            psAcm.__exit__(None, None, None)
            pscm = tc.tile_pool(name="ps", bufs=3, space="PSUM")
            ps = pscm.__enter__()
            p2cm = tc.tile_pool(name="p2", bufs=1)
            p2 = p2cm.__enter__()
            _ot_tiles = [[p2.tile([128, 512], BF16, name=f"ot{c}_{i}", tag=f"ot{c}_{i}")
                          for i in range(IT)] for c in range(NCH)]

            # ---- attention: q-chunk outer, two heads interleaved ----
            I32 = mybir.dt.int32
            AL = mybir.AluOpType

            def _s_part(h, ch, jb, ej):
                ti, pb = h // 2, 64 * (h % 2)
                pst = ps.tile([128, 512], F32, name="pst", tag="pst", bufs=3)
                nc.tensor.matmul(
                    pst[:],
                    kT[ti][pb:pb + 64, jb * 128:(jb + 1) * 128],
                    qT[ti][pb:pb + 64, ch * 512:(ch + 1) * 512])
                return pst

            def _e_part(h, ch, jb, ej, pst):
                pt = ptp.tile([128, 512], PV_DT, name="pt", tag="pt", bufs=5)
                nc.scalar.activation(pt[:], pst[:],
                                     mybir.ActivationFunctionType.Exp)
                if jb >= ej - 4:  # diagonal block: staircase mask
                    o = 128 * (jb - (ej - 4))
                    nc.vector.tensor_mul(pt[:], pt[:],
                                         mask[:, 384 - o:896 - o])
                return pt

            def _o_part(h, ch, jb, ej, po, pt):
                nc.tensor.matmul(
                    po[0:65, :],
                    vg[jb][:, h * (DH + 1):(h + 1) * (DH + 1)],
                    pt[:], start=(jb == 0), stop=(jb == ej - 1))

            def _norm(h, ch, po):
                ti, pb = h // 2, 64 * (h % 2)
                # 1/x via int bit-hack + one Newton step, all on DVE
                y0 = stp.tile([65, 512], F32, name="y0", tag="y0")
                nc.vector.tensor_scalar(
                    y0[64:65, :].bitcast(I32), po[64:65, :].bitcast(I32),
                    -1, 0x7EF311C3, op0=AL.mult, op1=AL.add)
                m = stp.tile([65, 512], F32, name="m", tag="m2")
                nc.vector.tensor_mul(m[64:65, :], po[64:65, :], y0[64:65, :])
                nc.vector.tensor_scalar(m[64:65, :], m[64:65, :],
                                        -1.0, 2.0, op0=AL.mult, op1=AL.add)
                rec = stp.tile([65, 512], PV_DT, name="rec", tag="nrm")
                nc.vector.tensor_mul(rec[64:65, :], y0[64:65, :], m[64:65, :])
                prep = ps.tile([64, 512], F32, name="prep", tag="pmix", bufs=2)
                nc.tensor.matmul(prep[:], ones1[64:65, :], rec[64:65, :])
                reps = stp.tile([64, 512], F32, name="reps", tag="so")
                nc.vector.tensor_copy(reps[:], prep[:])
                onorm = stp.tile([64, 512], PV_DT, name="onorm", tag="nrm")
                nc.vector.tensor_mul(onorm[:], po[0:64, :], reps[:])
                ot = _ot_tiles[ch][ti]
                if pb == 0:
                    nc.vector.tensor_copy(ot[0:64, :], onorm[:])
                else:
                    nc.sync.dma_start(ot[64:128, :], onorm[:])

            for ch in range(NCH):
                ej = 4 * (ch + 1)
                for hp in range(0, G, 2):
                    po0 = ps.tile([128, 512], F32, name="po0", tag="pot", bufs=3)
                    po1 = ps.tile([128, 512], F32, name="po1", tag="pot", bufs=3)
                    for jb in range(ej):
                        ps0 = _s_part(hp, ch, jb, ej)
                        ps1 = _s_part(hp + 1, ch, jb, ej)
                        pt0 = _e_part(hp, ch, jb, ej, ps0)
                        pt1 = _e_part(hp + 1, ch, jb, ej, ps1)
                        _o_part(hp, ch, jb, ej, po0, pt0)
                        _o_part(hp + 1, ch, jb, ej, po1, pt1)
                    _norm(hp, ch, po0)
                    _norm(hp + 1, ch, po1)

            # ---- output projection per chunk ----
            wot = [p2.tile([128, DIM], BF16, name=f"wo{i}", tag=f"wo{i}") for i in range(IT)]
            for i in range(IT):
                nc.sync.dma_start(wot[i][:], wo[i * 128:(i + 1) * 128, :])
            for ch in range(NCH):
                for rb in range(4):
                    for nco in range(2):
                        pf = ps.tile([128, 512], F32, name="pf", tag="pmix", bufs=2)
                        for i in range(IT):
                            nc.tensor.matmul(
                                pf[:],
                                _ot_tiles[ch][i][:, rb * 128:(rb + 1) * 128],
                                wot[i][:, nco * 512:(nco + 1) * 512],
                                start=(i == 0), stop=(i == IT - 1))
                        so = stp.tile([128, 512], F32, tag="so")
                        nc.vector.tensor_copy(so[:], pf[:])
                        nc.sync.dma_start(
                            out[ch * 512 + rb * 128:ch * 512 + (rb + 1) * 128,
                                nco * 512:(nco + 1) * 512], so[:])
            p2cm.__exit__(None, None, None)
            pscm.__exit__(None, None, None)

    nc.compile()
    return nc


def kernel(x, w_qkv, w_out, b_out):
    if "nc" not in _CACHE:
        _CACHE["nc"] = _build()
    nc = _CACHE["nc"]

    x = np.asarray(x, np.float32)
    w_qkv = np.asarray(w_qkv, np.float32)
    w_out = np.asarray(w_out, np.float32)
    b_out = np.asarray(b_out, np.float32)

    mask = (np.arange(128)[:, None] <= (np.arange(896)[None, :] - 384)).astype(np.float32)
    in_maps = []
    for c in range(8):
        b, g = c // 2, c % 2
        sl = slice(g * GI, (g + 1) * GI)
        in_maps.append(dict(
            xT=np.ascontiguousarray(x[b].T).astype(ml_dtypes.bfloat16),
            wq=(np.ascontiguousarray(w_qkv[:, sl]) * np.float32(SCALE)).astype(ml_dtypes.bfloat16),
            wk=np.ascontiguousarray(w_qkv[:, 1024 + g * GI:1024 + (g + 1) * GI]).astype(ml_dtypes.bfloat16),
            wv=np.ascontiguousarray(w_qkv[:, 2048 + g * GI:2048 + (g + 1) * GI]).astype(ml_dtypes.bfloat16),
            wo=np.ascontiguousarray(w_out[sl, :]).astype(ml_dtypes.bfloat16),
            msk=mask.astype(ml_dtypes.bfloat16),
            on1=np.ones((128, 64), ml_dtypes.bfloat16),
            onv=np.ones((128, G), ml_dtypes.bfloat16),
        ))
    res = None
    for attempt in range(3):
        try:
            res = run_bass_kernel_spmd(nc, in_maps, core_ids=list(range(8)))
            break
        except Exception:
            if attempt == 2:
                raise
            time.sleep(10)
    _CACHE["res"] = res
    outs = [res.results[c]["out"] for c in range(8)]
    full = np.empty((B, N, DIM), np.float32)
    for b in range(B):
        full[b] = outs[2 * b] + outs[2 * b + 1] + b_out[None, :]
    return full



# revision 8
# speedup vs baseline: 1.3365x; 1.3365x over previous
import sys, time

sys.path.insert(0, "/opt/trn_rl_repo")
import numpy as np
import ml_dtypes
from concourse import bass, bacc, tile, mybir
from concourse.bass_utils import run_bass_kernel_spmd

F32 = mybir.dt.float32
BF16 = mybir.dt.bfloat16
I32 = mybir.dt.int32
AL = mybir.AluOpType

B, N, DIM = 4, 2048, 1024
HEADS, DH = 16, 64
G = 8          # heads per core
GI = G * DH    # 512 = inner width per core
SCALE = DH ** -0.5
NB = N // 128   # 16 j-blocks
NCH = N // 512  # 4 q-chunks
DT = DIM // 128  # 8 dim tiles
IT = GI // 128   # 4 inner tiles

_CACHE = {}


def _build():
    nc = bacc.Bacc(None, target_bir_lowering=False)
    xT = nc.declare_dram_parameter("xT", [DIM, N], BF16, isOutput=False)
    wq = nc.declare_dram_parameter("wq", [DIM, GI], BF16, isOutput=False)
    wk = nc.declare_dram_parameter("wk", [DIM, GI], BF16, isOutput=False)
    wv = nc.declare_dram_parameter("wv", [DIM, GI], BF16, isOutput=False)
    wo = nc.declare_dram_parameter("wo", [GI, DIM], BF16, isOutput=False)
    msk = nc.declare_dram_parameter("msk", [128, 2048], BF16, isOutput=False)
    ese = nc.declare_dram_parameter("ese", [97, 256], BF16, isOutput=False)
    onv = nc.declare_dram_parameter("onv", [128, G], BF16, isOutput=False)
    out = nc.declare_dram_parameter("out", [DIM, N], F32, isOutput=True)

    with tile.TileContext(nc) as tc:
        with (
            nc.allow_low_precision(reason="attention in bf16; rel-err gate 2e-2"),
            tc.tile_pool(name="big", bufs=1) as big,
            tc.tile_pool(name="pt", bufs=6) as ptp,
            tc.tile_pool(name="st", bufs=2) as stp,
            tc.tile_pool(name="ps", bufs=2, space="PSUM") as ps,
        ):
            # ---------------- persistent SBUF ----------------
            qT = [big.tile([128, N], BF16, name=f"qT{i}", tag=f"qT{i}") for i in range(IT)]
            kT = [big.tile([128, N], BF16, name=f"kT{i}", tag=f"kT{i}") for i in range(IT)]
            vg = [big.tile([128, G * (DH + 1)], BF16, name=f"v{r}", tag=f"v{r}") for r in range(NB)]
            mask = big.tile([128, 2048], BF16, name="mask", tag="mask")
            esel = big.tile([97, 256], BF16, name="esel", tag="esel")
            ot = [[big.tile([128, 512], BF16, name=f"ot{c}_{i}", tag=f"ot{c}_{i}") for i in range(IT)]
                  for c in range(NCH)]

            p1cm = tc.tile_pool(name="p1", bufs=1)
            p1 = p1cm.__enter__()
            xTt = [[p1.tile([128, 512], BF16, name=f"xT{d}_{rc}", tag=f"xT{d}_{rc}") for rc in range(4)]
                   for d in range(DT)]
            wqt = [p1.tile([128, GI], BF16, name=f"wq{d}", tag=f"wq{d}") for d in range(DT)]
            wkt = [p1.tile([128, GI], BF16, name=f"wk{d}", tag=f"wk{d}") for d in range(DT)]
            wvt = [p1.tile([128, GI], BF16, name=f"wv{d}", tag=f"wv{d}") for d in range(DT)]
            wot = [p1.tile([128, DIM], BF16, name=f"wo{i}", tag=f"wo{i}") for i in range(IT)]

            # warm the ACT exp table early (overlaps with phase-1 matmuls)
            dmy = stp.tile([32, 32], F32, name="dmy", tag="dmy")
            nc.vector.memset(dmy[:], 0.0)
            nc.scalar.activation(dmy[:], dmy[:], mybir.ActivationFunctionType.Exp)

            # DMAs: wk + xT rc0 first on the sync queue (first compute group's
            # operands); the rest dispatched from the idle ACT hwdge queue
            for d in range(DT):
                nc.sync.dma_start(wkt[d][:], wk[d * 128:(d + 1) * 128, :])
                nc.sync.dma_start(xTt[d][0][:], xT[d * 128:(d + 1) * 128, 0:512])
            for rc in range(1, 4):
                for d in range(DT):
                    nc.sync.dma_start(xTt[d][rc][:],
                                      xT[d * 128:(d + 1) * 128, rc * 512:(rc + 1) * 512])
            for d in range(DT):
                nc.scalar.dma_start(wvt[d][:], wv[d * 128:(d + 1) * 128, :])
            for d in range(DT):
                nc.scalar.dma_start(wqt[d][:], wq[d * 128:(d + 1) * 128, :])
            nc.scalar.dma_start(mask[:], msk[:])
            nc.scalar.dma_start(esel[:], ese[:])
            for r in range(NB):
                dst = vg[r][:].rearrange("p (h c) -> p h c", c=DH + 1)[:, :, DH:DH + 1]
                nc.scalar.dma_start(dst, onv[:].rearrange("p (h c) -> p h c", c=1))
            for i in range(IT):
                nc.scalar.dma_start(wot[i][:], wo[i * 128:(i + 1) * 128, :])

            # ---------------- projections ----------------
            evac_flip = [0]

            def evac2(dst, src):
                # phase-1 only: alternate PSUM evacuations between DVE and ACT
                if evac_flip[0] % 2 == 0:
                    nc.vector.tensor_copy(dst, src)
                else:
                    nc.scalar.copy(dst, src)
                evac_flip[0] += 1

            def kproj_group(it, rc):
                pk = ps.tile([128, 512], F32, name="pmix", tag="pmix")
                for d in range(DT):
                    nc.tensor.matmul(pk[:], wkt[d][:, it * 128:(it + 1) * 128],
                                     xTt[d][rc][:], start=(d == 0), stop=(d == DT - 1))
                evac2(kT[it][:, rc * 512:(rc + 1) * 512], pk[:])

            def vproj_group(r):
                pv = ps.tile([128, 512], F32, name="pmix", tag="pmix")
                for d in range(DT):
                    nc.tensor.matmul(pv[:], xTt[d][r // 4][:, (r % 4) * 128:(r % 4 + 1) * 128],
                                     wvt[d][:], start=(d == 0), stop=(d == DT - 1))
                dst = vg[r][:].rearrange("p (h c) -> p h c", c=DH + 1)[:, :, 0:DH]
                nc.vector.tensor_copy(dst, pv[:].rearrange("p (h c) -> p h c", c=DH))

            def qproj_group(it, rc, dve_only=False):
                pq = ps.tile([128, 512], F32, name="pmix", tag="pmix")
                for d in range(DT):
                    nc.tensor.matmul(pq[:], wqt[d][:, it * 128:(it + 1) * 128],
                                     xTt[d][rc][:], start=(d == 0), stop=(d == DT - 1))
                if dve_only:
                    nc.vector.tensor_copy(qT[it][:, rc * 512:(rc + 1) * 512], pq[:])
                else:
                    evac2(qT[it][:, rc * 512:(rc + 1) * 512], pq[:])

            for rc in range(4):
                for it in range(IT):
                    kproj_group(it, rc)
            for r in range(NB):
                vproj_group(r)
            for it in range(IT):
                qproj_group(it, 0)

            # ---------------- out-projection (transposed) ----------------
            def outproj_group(ch, db):
                # out^T[db-block, i-chunk]: lhsT is the stable weight tile so
                # the PE's LDWEIGHTS pull-ahead never reads freshly-written data
                pf = ps.tile([128, 512], F32, name="pmix", tag="pmix")
                for i in range(IT):
                    nc.tensor.matmul(pf[:], wot[i][:, db * 128:(db + 1) * 128],
                                     ot[ch][i][:],
                                     start=(i == 0), stop=(i == IT - 1))
                so = stp.tile([128, 512], F32, name="so", tag="so")
                nc.vector.tensor_copy(so[:], pf[:])
                nc.sync.dma_start(
                    out[db * 128:(db + 1) * 128,
                        ch * 512:(ch + 1) * 512], so[:])

            # ---------------- attention ----------------
            for ch in range(NCH):
                ej = 4 * (ch + 1)
                d4a = stp.tile([97, 512], F32, name="d4", tag="d4")
                d4b = stp.tile([97, 512], F32, name="d4", tag="d4")
                nc.vector.memset(d4a[:], 1.0)
                nc.vector.memset(d4b[:], 1.0)
                d4 = [d4a, d4a, d4b, d4b]

                for hp in range(4):
                    hA, hB = 2 * hp, 2 * hp + 1
                    po0 = ps.tile([65, 512], F32, name="pot", tag="pot")
                    po1 = ps.tile([65, 512], F32, name="pot", tag="pot")

                    def s_blk(jb):
                        # S^T pair for one j-block: two row-tiled K=64 matmuls
                        sA = ps.tile([128, 512], F32, name="sup", tag="sup", bufs=4)
                        sB = ps.tile([128, 512], F32, name="sup", tag="sup", bufs=4)
                        nc.tensor.matmul(
                            sA[:], kT[hp][0:64, jb * 128:(jb + 1) * 128],
                            qT[hp][0:64, ch * 512:(ch + 1) * 512])
                        nc.tensor.matmul(
                            sB[:], kT[hp][64:128, jb * 128:(jb + 1) * 128],
                            qT[hp][64:128, ch * 512:(ch + 1) * 512])
                        return sA, sB

                    nxt = s_blk(0)
                    for jb in range(ej):
                        sA, sB = nxt
                        ptA = ptp.tile([128, 512], BF16, name="pt", tag="pt")
                        ptB = ptp.tile([128, 512], BF16, name="pt", tag="pt")
                        nc.scalar.activation(ptA[:], sA[:],
                                             mybir.ActivationFunctionType.Exp)
                        nc.scalar.activation(ptB[:], sB[:],
                                             mybir.ActivationFunctionType.Exp)
                        if jb + 1 < ej:
                            nxt = s_blk(jb + 1)
                        if jb >= ej - 4:  # diagonal blocks: staircase mask
                            r = jb - (ej - 4)
                            nc.vector.tensor_mul(ptA[:], ptA[:],
                                                 mask[:, 512 * r:512 * r + 512])
                            nc.vector.tensor_mul(ptB[:], ptB[:],
                                                 mask[:, 512 * r:512 * r + 512])
                        st_, sp_ = (jb == 0), (jb == ej - 1)
                        nc.tensor.matmul(
                            po0[0:65, :], vg[jb][:, hA * (DH + 1):(hA + 1) * (DH + 1)],
                            ptA[:], start=st_, stop=sp_)
                        nc.tensor.matmul(
                            po1[0:65, :], vg[jb][:, hB * (DH + 1):(hB + 1) * (DH + 1)],
                            ptB[:], start=st_, stop=sp_)

                    # D rows -> 32-aligned slots; O -> ot (unnormalized)
                    rA, rB = 32 * (hA % 4), 32 * (hB % 4)
                    nc.vector.tensor_copy(d4[hp][rA:rA + 1, :], po0[64:65, :])
                    nc.vector.tensor_copy(d4[hp][rB:rB + 1, :], po1[64:65, :])
                    nc.vector.tensor_copy(ot[ch][hp][0:64, :], po0[0:64, :])
                    nc.vector.tensor_copy(ot[ch][hp][64:128, :], po1[0:64, :])
                    if ch + 1 < NCH:
                        qproj_group(hp, ch + 1, dve_only=True)

                # ---- batched norm for this chunk ----
                for x in range(2):
                    dd = d4a if x == 0 else d4b
                    y0 = stp.tile([97, 512], F32, name="y0", tag="y0")
                    nc.vector.reciprocal_approx_fast(y0[:], dd[:])
                    rec = stp.tile([97, 512], BF16, name="rec", tag="rec")
                    nc.vector.tensor_copy(rec[:], y0[:])
                    for t in range(2):
                        ti = 2 * x + t
                        prep = ps.tile([128, 512], F32, name="pmix", tag="pmix")
                        nc.tensor.matmul(prep[:], esel[:, 128 * t:128 * (t + 1)],
                                         rec[:], start=True, stop=True)
                        nc.vector.tensor_mul(ot[ch][ti][:], ot[ch][ti][:], prep[:])

                # out-proj for this chunk: filler for the scheduler during the
                # next chunk's ACT-bound attention
                for db in range(DT):
                    outproj_group(ch, db)

            p1cm.__exit__(None, None, None)

    nc.compile()
    return nc


def kernel(x, w_qkv, w_out, b_out):
    if "nc" not in _CACHE:
        _CACHE["nc"] = _build()
    nc = _CACHE["nc"]

    x = np.asarray(x, np.float32)
    w_qkv = np.asarray(w_qkv, np.float32)
    w_out = np.asarray(w_out, np.float32)
    b_out = np.asarray(b_out, np.float32)

    # staircase masks for the 4 diagonal block offsets:
    # mask_r[p, i] = 1 if p <= i - 128r ; layout [r0 | r1 | r2 | r3]
    p = np.arange(128)[:, None]
    i = np.arange(512)[None, :]
    msk2 = np.concatenate(
        [(p <= i - 128 * r).astype(np.float32) for r in range(4)], axis=1)

    # selector for denominator broadcast: [97, 256]
    ese = np.zeros((97, 256), np.float32)
    ese[0, 0:64] = 1.0      # even ti: head rows 0 (p<64), 32 (p>=64)
    ese[32, 64:128] = 1.0
    ese[64, 128:192] = 1.0  # odd ti: rows 64, 96
    ese[96, 192:256] = 1.0

    in_maps = []
    for c in range(8):
        b, g = c // 2, c % 2
        sl = slice(g * GI, (g + 1) * GI)
        in_maps.append(dict(
            xT=np.ascontiguousarray(x[b].T).astype(ml_dtypes.bfloat16),
            wq=(np.ascontiguousarray(w_qkv[:, sl]) * np.float32(SCALE)).astype(ml_dtypes.bfloat16),
            wk=np.ascontiguousarray(w_qkv[:, 1024 + g * GI:1024 + (g + 1) * GI]).astype(ml_dtypes.bfloat16),
            wv=np.ascontiguousarray(w_qkv[:, 2048 + g * GI:2048 + (g + 1) * GI]).astype(ml_dtypes.bfloat16),
            wo=np.ascontiguousarray(w_out[sl, :]).astype(ml_dtypes.bfloat16),
            msk=msk2.astype(ml_dtypes.bfloat16),
            ese=ese.astype(ml_dtypes.bfloat16),
            onv=np.ones((128, G), ml_dtypes.bfloat16),
        ))
    res = None
    for attempt in range(3):
        try:
            run_bass_kernel_spmd(nc, in_maps, core_ids=list(range(8)))  # warmup
            res = run_bass_kernel_spmd(nc, in_maps, core_ids=list(range(8)))
            break
        except Exception:
            if attempt == 2:
                raise
            time.sleep(10)
    _CACHE["res"] = res
    outs = [res.results[c]["out"] for c in range(8)]
    full = np.empty((B, N, DIM), np.float32)
    for b in range(B):
        full[b] = (outs[2 * b] + outs[2 * b + 1]).T + b_out[None, :]
    return full


# revision 9
# speedup vs baseline: 1.3904x; 1.0403x over previous
import sys, time

sys.path.insert(0, "/opt/trn_rl_repo")
import numpy as np
import ml_dtypes
from concourse import bass, bacc, tile, mybir
from concourse.bass_utils import run_bass_kernel_spmd

F32 = mybir.dt.float32
BF16 = mybir.dt.bfloat16
I32 = mybir.dt.int32
AL = mybir.AluOpType

B, N, DIM = 4, 2048, 1024
HEADS, DH = 16, 64
G = 8          # heads per core
GI = G * DH    # 512 = inner width per core
SCALE = DH ** -0.5
NB = N // 128   # 16 j-blocks
NCH = N // 512  # 4 q-chunks
DT = DIM // 128  # 8 dim tiles
IT = GI // 128   # 4 inner tiles

_CACHE = {}


def _build():
    nc = bacc.Bacc(None, target_bir_lowering=False)
    xT = nc.declare_dram_parameter("xT", [DIM, N], BF16, isOutput=False)
    wq = nc.declare_dram_parameter("wq", [DIM, GI], BF16, isOutput=False)
    wk = nc.declare_dram_parameter("wk", [DIM, GI], BF16, isOutput=False)
    wv = nc.declare_dram_parameter("wv", [DIM, GI], BF16, isOutput=False)
    wo = nc.declare_dram_parameter("wo", [GI, DIM], BF16, isOutput=False)
    msk = nc.declare_dram_parameter("msk", [128, 2048], BF16, isOutput=False)
    ese = nc.declare_dram_parameter("ese", [97, 256], BF16, isOutput=False)
    onv = nc.declare_dram_parameter("onv", [128, G], BF16, isOutput=False)
    out = nc.declare_dram_parameter("out", [DIM, N], BF16, isOutput=True)

    with tile.TileContext(nc) as tc:
        with (
            nc.allow_low_precision(reason="attention in bf16; rel-err gate 2e-2"),
            tc.tile_pool(name="big", bufs=1) as big,
            tc.tile_pool(name="pt", bufs=4) as ptp,
            tc.tile_pool(name="st", bufs=2) as stp,
            tc.tile_pool(name="ps", bufs=2, space="PSUM") as ps,
        ):
            # ---------------- persistent SBUF ----------------
            qT = [big.tile([128, N], BF16, name=f"qT{i}", tag=f"qT{i}") for i in range(IT)]
            kT = [big.tile([128, N], BF16, name=f"kT{i}", tag=f"kT{i}") for i in range(IT)]
            vg = [big.tile([128, G * (DH + 1)], BF16, name=f"v{r}", tag=f"v{r}") for r in range(NB)]
            mask = big.tile([128, 2048], BF16, name="mask", tag="mask")
            esel = big.tile([97, 256], BF16, name="esel", tag="esel")
            ot = [[big.tile([128, 512], BF16, name=f"ot{c}_{i}", tag=f"ot{c}_{i}") for i in range(IT)]
                  for c in range(NCH)]

            p1cm = tc.tile_pool(name="p1", bufs=1)
            p1 = p1cm.__enter__()
            xTt = [[p1.tile([128, 512], BF16, name=f"xT{d}_{rc}", tag=f"xT{d}_{rc}") for rc in range(4)]
                   for d in range(DT)]
            wqt = [p1.tile([128, GI], BF16, name=f"wq{d}", tag=f"wq{d}") for d in range(DT)]
            wkt = [p1.tile([128, GI], BF16, name=f"wk{d}", tag=f"wk{d}") for d in range(DT)]
            wvt = [p1.tile([128, GI], BF16, name=f"wv{d}", tag=f"wv{d}") for d in range(DT)]
            wot = [p1.tile([128, DIM], BF16, name=f"wo{i}", tag=f"wo{i}") for i in range(IT)]

            # warm the ACT exp table + keep the PE busy (HAM warm) while the
            # first input DMAs land
            dmy = stp.tile([32, 32], F32, name="dmy", tag="dmy")
            nc.vector.memset(dmy[:], 0.0)
            nc.scalar.activation(dmy[:], dmy[:], mybir.ActivationFunctionType.Exp)
            dmw = p1.tile([128, 512], BF16, name="dmw", tag="dmw")
            nc.vector.memset(dmw[:], 0.0)
            for w in range(28):
                pw = ps.tile([128, 512], F32, name="pmix", tag="pmix")
                nc.tensor.matmul(pw[:], dmw[:, 0:128], dmw[:], start=True, stop=True)

            # DMAs: wk + xT rc0 first (first compute group's operands), xT
            # split across the two hwdge queues, weights on the ACT queue
            for d in range(DT):
                nc.sync.dma_start(wkt[d][:], wk[d * 128:(d + 1) * 128, :])
                nc.sync.dma_start(xTt[d][0][:], xT[d * 128:(d + 1) * 128, 0:512])
            for d in range(DT):
                nc.scalar.dma_start(wvt[d][:], wv[d * 128:(d + 1) * 128, :])
                nc.scalar.dma_start(wqt[d][:], wq[d * 128:(d + 1) * 128, :])
            for r in range(4):
                dst = vg[r][:].rearrange("p (h c) -> p h c", c=DH + 1)[:, :, DH:DH + 1]
                nc.scalar.dma_start(dst, onv[:].rearrange("p (h c) -> p h c", c=1))
            for rc in range(1, 4):
                for d in range(DT):
                    q_ = nc.sync if (d % 2 == 0) else nc.scalar
                    q_.dma_start(xTt[d][rc][:],
                                 xT[d * 128:(d + 1) * 128, rc * 512:(rc + 1) * 512])
            nc.scalar.dma_start(mask[:], msk[:])
            nc.scalar.dma_start(esel[:], ese[:])
            for r in range(4, NB):
                dst = vg[r][:].rearrange("p (h c) -> p h c", c=DH + 1)[:, :, DH:DH + 1]
                nc.scalar.dma_start(dst, onv[:].rearrange("p (h c) -> p h c", c=1))
            for i in range(IT):
                nc.scalar.dma_start(wot[i][:], wo[i * 128:(i + 1) * 128, :])

            # ---------------- projections ----------------
            evac_flip = [0]

            def evac2(dst, src):
                # alternate PSUM evacuations between DVE and ACT (phase 1 only)
                if evac_flip[0] % 2 == 0:
                    nc.vector.tensor_copy(dst, src)
                else:
                    nc.scalar.copy(dst, src)
                evac_flip[0] += 1

            def kproj_group(it, rc):
                pk = ps.tile([128, 512], F32, name="pmix", tag="pmix")
                for d in range(DT):
                    nc.tensor.matmul(pk[:], wkt[d][:, it * 128:(it + 1) * 128],
                                     xTt[d][rc][:], start=(d == 0), stop=(d == DT - 1))
                evac2(kT[it][:, rc * 512:(rc + 1) * 512], pk[:])

            def vproj_group(r):
                pv = ps.tile([128, 512], F32, name="pmix", tag="pmix")
                for d in range(DT):
                    nc.tensor.matmul(pv[:], xTt[d][r // 4][:, (r % 4) * 128:(r % 4 + 1) * 128],
                                     wvt[d][:], start=(d == 0), stop=(d == DT - 1))
                dst = vg[r][:].rearrange("p (h c) -> p h c", c=DH + 1)[:, :, 0:DH]
                nc.vector.tensor_copy(dst, pv[:].rearrange("p (h c) -> p h c", c=DH))

            def qproj_group(it, rc, dve_only=False):
                pq = ps.tile([128, 512], F32, name="pmix", tag="pmix")
                for d in range(DT):
                    nc.tensor.matmul(pq[:], wqt[d][:, it * 128:(it + 1) * 128],
                                     xTt[d][rc][:], start=(d == 0), stop=(d == DT - 1))
                if dve_only:
                    nc.vector.tensor_copy(qT[it][:, rc * 512:(rc + 1) * 512], pq[:])
                else:
                    evac2(qT[it][:, rc * 512:(rc + 1) * 512], pq[:])

            # ---------------- out-projection (transposed) ----------------
            def outproj_group(ch, db):
                # out^T[db-block, i-chunk]: lhsT is the stable weight tile so
                # the PE's LDWEIGHTS pull-ahead never reads freshly-written data
                pf = ps.tile([128, 512], F32, name="pmix", tag="pmix")
                for i in range(IT):
                    nc.tensor.matmul(pf[:], wot[i][:, db * 128:(db + 1) * 128],
                                     ot[ch][i][:],
                                     start=(i == 0), stop=(i == IT - 1))
                so = stp.tile([128, 512], BF16, name="so", tag="so")
                nc.vector.tensor_copy(so[:], pf[:])
                nc.sync.dma_start(
                    out[db * 128:(db + 1) * 128,
                        ch * 512:(ch + 1) * 512], so[:])

            # ---------------- attention ----------------
            def attention_chunk(ch):
                ej = 4 * (ch + 1)
                ns = ej // 2
                d4a = stp.tile([97, 512], F32, name="d4", tag="d4")
                d4b = stp.tile([97, 512], F32, name="d4", tag="d4")
                nc.vector.memset(d4a[:], 1.0)
                nc.vector.memset(d4b[:], 1.0)
                d4 = [d4a, d4a, d4b, d4b]

                for hp in range(4):
                    hA, hB = 2 * hp, 2 * hp + 1
                    po0 = ps.tile([65, 512], F32, name="pot", tag="pot")
                    po1 = ps.tile([65, 512], F32, name="pot", tag="pot")

                    def s_mm(s):
                        # S^T pair for a 2-block supertile: row-tiled K=64 MMs
                        supA = ps.tile([128, 1024], F32, name="sup", tag="sup")
                        supB = ps.tile([128, 1024], F32, name="sup", tag="sup")
                        for half, jb in ((0, 2 * s), (1, 2 * s + 1)):
                            sl = slice(512 * half, 512 * half + 512)
                            nc.tensor.matmul(
                                supA[:, sl], kT[hp][0:64, jb * 128:(jb + 1) * 128],
                                qT[hp][0:64, ch * 512:(ch + 1) * 512])
                            nc.tensor.matmul(
                                supB[:, sl], kT[hp][64:128, jb * 128:(jb + 1) * 128],
                                qT[hp][64:128, ch * 512:(ch + 1) * 512])
                        return supA, supB

                    nxt = s_mm(0)
                    for s in range(ns):
                        supA, supB = nxt
                        ptA = ptp.tile([128, 1024], BF16, name="pt", tag="pt")
                        ptB = ptp.tile([128, 1024], BF16, name="pt", tag="pt")
                        nc.scalar.activation(ptA[:], supA[:],
                                             mybir.ActivationFunctionType.Exp)
                        nc.scalar.activation(ptB[:], supB[:],
                                             mybir.ActivationFunctionType.Exp)
                        if s + 1 < ns:
                            nxt = s_mm(s + 1)
                        if s >= ns - 2:  # diagonal supers: staircase mask
                            msl = slice(0, 1024) if s == ns - 2 else slice(1024, 2048)
                            nc.vector.tensor_mul(ptA[:], ptA[:], mask[:, msl])
                            nc.vector.tensor_mul(ptB[:], ptB[:], mask[:, msl])
                        for half, jb in ((0, 2 * s), (1, 2 * s + 1)):
                            sl = slice(512 * half, 512 * half + 512)
                            st_ = (s == 0 and half == 0)
                            sp_ = (s == ns - 1 and half == 1)
                            nc.tensor.matmul(
                                po0[0:65, :], vg[jb][:, hA * (DH + 1):(hA + 1) * (DH + 1)],
                                ptA[:, sl], start=st_, stop=sp_)
                            nc.tensor.matmul(
                                po1[0:65, :], vg[jb][:, hB * (DH + 1):(hB + 1) * (DH + 1)],
                                ptB[:, sl], start=st_, stop=sp_)

                    # D rows -> 32-aligned slots; O -> ot (unnormalized)
                    rA, rB = 32 * (hA % 4), 32 * (hB % 4)
                    nc.vector.tensor_copy(d4[hp][rA:rA + 1, :], po0[64:65, :])
                    nc.vector.tensor_copy(d4[hp][rB:rB + 1, :], po1[64:65, :])
                    nc.vector.tensor_copy(ot[ch][hp][0:64, :], po0[0:64, :])
                    nc.vector.tensor_copy(ot[ch][hp][64:128, :], po1[0:64, :])
                    if ch + 1 < NCH:
                        qproj_group(hp, ch + 1, dve_only=True)

                # ---- batched norm for this chunk ----
                for x in range(2):
                    dd = d4a if x == 0 else d4b
                    y0 = stp.tile([97, 512], F32, name="y0", tag="y0")
                    nc.vector.reciprocal_approx_fast(y0[:], dd[:])
                    rec = stp.tile([97, 512], BF16, name="rec", tag="rec")
                    nc.vector.tensor_copy(rec[:], y0[:])
                    for t in range(2):
                        ti = 2 * x + t
                        prep = ps.tile([128, 512], F32, name="pmix", tag="pmix")
                        nc.tensor.matmul(prep[:], esel[:, 128 * t:128 * (t + 1)],
                                         rec[:], start=True, stop=True)
                        nc.vector.tensor_mul(ot[ch][ti][:], ot[ch][ti][:], prep[:])

                # out-proj: filler for the next chunk's ACT-bound attention
                for db in range(DT):
                    outproj_group(ch, db)

            # phase 1 minimal prefix: everything attention ch0 needs
            for it in range(IT):
                kproj_group(it, 0)
            for r in range(4):
                vproj_group(r)
            for it in range(IT):
                qproj_group(it, 0)

            attention_chunk(0)

            # remaining projections: emitted after ch0 so they fill its
            # ACT-bound windows; all complete before the chunks that need them
            for rc in range(1, 4):
                for it in range(IT):
                    kproj_group(it, rc)
            for r in range(4, NB):
                vproj_group(r)

            for ch in range(1, NCH):
                attention_chunk(ch)

            p1cm.__exit__(None, None, None)

    nc.compile()
    return nc


def kernel(x, w_qkv, w_out, b_out):
    if "nc" not in _CACHE:
        _CACHE["nc"] = _build()
    nc = _CACHE["nc"]

    x = np.asarray(x, np.float32)
    w_qkv = np.asarray(w_qkv, np.float32)
    w_out = np.asarray(w_out, np.float32)
    b_out = np.asarray(b_out, np.float32)

    # staircase masks for the 4 diagonal block offsets:
    # mask_r[p, i] = 1 if p <= i - 128r ; layout [r0 | r1 | r2 | r3]
    p = np.arange(128)[:, None]
    i = np.arange(512)[None, :]
    msk2 = np.concatenate(
        [(p <= i - 128 * r).astype(np.float32) for r in range(4)], axis=1)

    # selector for denominator broadcast: [97, 256]
    ese = np.zeros((97, 256), np.float32)
    ese[0, 0:64] = 1.0      # even ti: head rows 0 (p<64), 32 (p>=64)
    ese[32, 64:128] = 1.0
    ese[64, 128:192] = 1.0  # odd ti: rows 64, 96
    ese[96, 192:256] = 1.0

    in_maps = []
    for c in range(8):
        b, g = c // 2, c % 2
        sl = slice(g * GI, (g + 1) * GI)
        in_maps.append(dict(
            xT=np.ascontiguousarray(x[b].T).astype(ml_dtypes.bfloat16),
            wq=(np.ascontiguousarray(w_qkv[:, sl]) * np.float32(SCALE)).astype(ml_dtypes.bfloat16),
            wk=np.ascontiguousarray(w_qkv[:, 1024 + g * GI:1024 + (g + 1) * GI]).astype(ml_dtypes.bfloat16),
            wv=np.ascontiguousarray(w_qkv[:, 2048 + g * GI:2048 + (g + 1) * GI]).astype(ml_dtypes.bfloat16),
            wo=np.ascontiguousarray(w_out[sl, :]).astype(ml_dtypes.bfloat16),
            msk=msk2.astype(ml_dtypes.bfloat16),
            ese=ese.astype(ml_dtypes.bfloat16),
            onv=np.ones((128, G), ml_dtypes.bfloat16),
        ))
    res = None
    for attempt in range(3):
        try:
            run_bass_kernel_spmd(nc, in_maps, core_ids=list(range(8)))  # warmup
            res = run_bass_kernel_spmd(nc, in_maps, core_ids=list(range(8)))
            break
        except Exception:
            if attempt == 2:
                raise
            time.sleep(10)
    _CACHE["res"] = res
    outs = [np.asarray(res.results[c]["out"], np.float32) for c in range(8)]
    full = np.empty((B, N, DIM), np.float32)
    for b in range(B):
        full[b] = (outs[2 * b] + outs[2 * b + 1]).T + b_out[None, :]
    return full


# revision 10
# speedup vs baseline: 1.5461x; 1.1120x over previous
import sys, time

sys.path.insert(0, "/opt/trn_rl_repo")
import numpy as np
import ml_dtypes
from concourse import bass, bacc, tile, mybir
from concourse.bass_utils import run_bass_kernel_spmd

F32 = mybir.dt.float32
BF16 = mybir.dt.bfloat16
I32 = mybir.dt.int32
AL = mybir.AluOpType

B, N, DIM = 4, 2048, 1024
HEADS, DH = 16, 64
G = 8          # heads per core
GI = G * DH    # 512 = inner width per core
SCALE = DH ** -0.5
NB = N // 128   # 16 j-blocks
NCH = N // 512  # 4 q-chunks
DT = DIM // 128  # 8 dim tiles
IT = GI // 128   # 4 inner tiles

_CACHE = {}


def _build():
    nc = bacc.Bacc(None, target_bir_lowering=False)
    xT = nc.declare_dram_parameter("xT", [DIM, N], BF16, isOutput=False)
    wq = nc.declare_dram_parameter("wq", [DIM, GI], BF16, isOutput=False)
    wk = nc.declare_dram_parameter("wk", [DIM, GI], BF16, isOutput=False)
    wv = nc.declare_dram_parameter("wv", [DIM, GI], BF16, isOutput=False)
    wo = nc.declare_dram_parameter("wo", [GI, DIM], BF16, isOutput=False)
    msk = nc.declare_dram_parameter("msk", [128, 2048], BF16, isOutput=False)
    ese = nc.declare_dram_parameter("ese", [97, 256], BF16, isOutput=False)
    onv = nc.declare_dram_parameter("onv", [128, G], BF16, isOutput=False)
    out = nc.declare_dram_parameter("out", [DIM, N], BF16, isOutput=True)

    with tile.TileContext(nc) as tc:
        with (
            nc.allow_low_precision(reason="attention in bf16; rel-err gate 2e-2"),
            tc.tile_pool(name="big", bufs=1) as big,
            tc.tile_pool(name="pt", bufs=4) as ptp,
            tc.tile_pool(name="st", bufs=2) as stp,
            tc.tile_pool(name="ps", bufs=2, space="PSUM") as ps,
        ):
            # ---------------- persistent SBUF ----------------
            qT = [big.tile([128, N], BF16, name=f"qT{i}", tag=f"qT{i}") for i in range(IT)]
            kT = [big.tile([128, N], BF16, name=f"kT{i}", tag=f"kT{i}") for i in range(IT)]
            vg = [big.tile([128, G * (DH + 1)], BF16, name=f"v{r}", tag=f"v{r}") for r in range(NB)]
            mask = big.tile([128, 2048], BF16, name="mask", tag="mask")
            esel = big.tile([97, 256], BF16, name="esel", tag="esel")
            ot = [[big.tile([128, 512], BF16, name=f"ot{c}_{i}", tag=f"ot{c}_{i}") for i in range(IT)]
                  for c in range(NCH)]

            p1cm = tc.tile_pool(name="p1", bufs=1)
            p1 = p1cm.__enter__()
            xTb = [p1.tile([128, N], BF16, name=f"xT{d}", tag=f"xT{d}") for d in range(DT)]
            xTt = [[xTb[d][:, rc * 512:(rc + 1) * 512] for rc in range(4)] for d in range(DT)]
            wqb = p1.tile([128, DT * GI], BF16, name="wqb", tag="wqb")
            wkb = p1.tile([128, DT * GI], BF16, name="wkb", tag="wkb")
            wvb = p1.tile([128, DT * GI], BF16, name="wvb", tag="wvb")
            wqt = [wqb[:, d * GI:(d + 1) * GI] for d in range(DT)]
            wkt = [wkb[:, d * GI:(d + 1) * GI] for d in range(DT)]
            wvt = [wvb[:, d * GI:(d + 1) * GI] for d in range(DT)]
            wob = p1.tile([128, IT * DIM], BF16, name="wob", tag="wob")
            wot = [wob[:, i * DIM:(i + 1) * DIM] for i in range(IT)]

            # warm the ACT exp table + keep the PE busy (HAM warm) while the
            # first input DMAs land
            dmy = stp.tile([32, 32], F32, name="dmy", tag="dmy")
            nc.vector.memset(dmy[:], 0.0)
            nc.scalar.activation(dmy[:], dmy[:], mybir.ActivationFunctionType.Exp)
            dmw = p1.tile([128, 512], BF16, name="dmw", tag="dmw")
            nc.vector.memset(dmw[:], 0.0)
            for w in range(28):
                pw = ps.tile([128, 512], F32, name="pmix", tag="pmix")
                nc.tensor.matmul(pw[:], dmw[:, 0:128], dmw[:], start=True, stop=True)

            # DMAs: single coalesced descriptors, all on the sync queue so the
            # ACT queue stays free for compute. Critical-path order: wk, xT,
            # wv, wq, then the rest.
            nc.sync.dma_start(wkb[:].rearrange("p (d c) -> p d c", c=GI),
                              wk[:].rearrange("(d p) c -> p d c", p=128))
            for d in range(DT):
                nc.sync.dma_start(xTb[d][:], xT[d * 128:(d + 1) * 128, :])
            nc.sync.dma_start(wvb[:].rearrange("p (d c) -> p d c", c=GI),
                              wv[:].rearrange("(d p) c -> p d c", p=128))
            nc.sync.dma_start(wqb[:].rearrange("p (d c) -> p d c", c=GI),
                              wq[:].rearrange("(d p) c -> p d c", p=128))
            for r in range(4):
                dst = vg[r][:].rearrange("p (h c) -> p h c", c=DH + 1)[:, :, DH:DH + 1]
                nc.sync.dma_start(dst, onv[:].rearrange("p (h c) -> p h c", c=1))
            nc.sync.dma_start(mask[:], msk[:])
            nc.sync.dma_start(esel[:], ese[:])
            nc.sync.dma_start(wob[:].rearrange("p (i c) -> p i c", c=DIM),
                              wo[:].rearrange("(i p) c -> p i c", p=128))
            for r in range(4, NB):
                dst = vg[r][:].rearrange("p (h c) -> p h c", c=DH + 1)[:, :, DH:DH + 1]
                nc.sync.dma_start(dst, onv[:].rearrange("p (h c) -> p h c", c=1))

            # ---------------- projections ----------------
            evac_flip = [0]

            def evac2(dst, src):
                # alternate PSUM evacuations between DVE and ACT (phase 1 only)
                if evac_flip[0] % 2 == 0:
                    nc.vector.tensor_copy(dst, src)
                else:
                    nc.scalar.copy(dst, src)
                evac_flip[0] += 1

            def kproj_group(it, rc):
                pk = ps.tile([128, 512], F32, name="pmix", tag="pmix")
                for d in range(DT):
                    nc.tensor.matmul(pk[:], wkt[d][:, it * 128:(it + 1) * 128],
                                     xTt[d][rc], start=(d == 0), stop=(d == DT - 1))
                evac2(kT[it][:, rc * 512:(rc + 1) * 512], pk[:])

            def vproj_group(r):
                pv = ps.tile([128, 512], F32, name="pmix", tag="pmix")
                for d in range(DT):
                    nc.tensor.matmul(pv[:], xTb[d][:, r * 128:(r + 1) * 128],
                                     wvt[d], start=(d == 0), stop=(d == DT - 1))
                dst = vg[r][:].rearrange("p (h c) -> p h c", c=DH + 1)[:, :, 0:DH]
                nc.vector.tensor_copy(dst, pv[:].rearrange("p (h c) -> p h c", c=DH))

            def qproj_group(it, rc, dve_only=False):
                pq = ps.tile([128, 512], F32, name="pmix", tag="pmix")
                for d in range(DT):
                    nc.tensor.matmul(pq[:], wqt[d][:, it * 128:(it + 1) * 128],
                                     xTt[d][rc], start=(d == 0), stop=(d == DT - 1))
                if dve_only:
                    nc.vector.tensor_copy(qT[it][:, rc * 512:(rc + 1) * 512], pq[:])
                else:
                    evac2(qT[it][:, rc * 512:(rc + 1) * 512], pq[:])

            # ---------------- out-projection (transposed) ----------------
            def outproj_group(ch, db):
                # out^T[db-block, i-chunk]: lhsT is the stable weight tile so
                # the PE's LDWEIGHTS pull-ahead never reads freshly-written data
                pf = ps.tile([128, 512], F32, name="pmix", tag="pmix")
                for i in range(IT):
                    nc.tensor.matmul(pf[:], wot[i][:, db * 128:(db + 1) * 128],
                                     ot[ch][i][:],
                                     start=(i == 0), stop=(i == IT - 1))
                so = stp.tile([128, 512], BF16, name="so", tag="so")
                if db % 2 == 0:
                    nc.vector.tensor_copy(so[:], pf[:])
                else:
                    nc.scalar.copy(so[:], pf[:])
                q_ = nc.sync if db % 2 == 0 else nc.scalar
                q_.dma_start(
                    out[db * 128:(db + 1) * 128,
                        ch * 512:(ch + 1) * 512], so[:])

            # ---------------- attention ----------------
            def attention_chunk(ch):
                ej = 4 * (ch + 1)
                ns = ej // 2
                d4a = stp.tile([97, 512], F32, name="d4", tag="d4")
                d4b = stp.tile([97, 512], F32, name="d4", tag="d4")
                nc.vector.memset(d4a[:], 1.0)
                nc.vector.memset(d4b[:], 1.0)
                d4 = [d4a, d4a, d4b, d4b]

                for hp in range(4):
                    hA, hB = 2 * hp, 2 * hp + 1
                    po0 = ps.tile([65, 512], F32, name="pot", tag="pot")
                    po1 = ps.tile([65, 512], F32, name="pot", tag="pot")

                    def s_mm(s):
                        # S^T pair for a 2-block supertile: row-tiled K=64 MMs
                        supA = ps.tile([128, 1024], F32, name="sup", tag="sup")
                        supB = ps.tile([128, 1024], F32, name="sup", tag="sup")
                        for half, jb in ((0, 2 * s), (1, 2 * s + 1)):
                            sl = slice(512 * half, 512 * half + 512)
                            nc.tensor.matmul(
                                supA[:, sl], kT[hp][0:64, jb * 128:(jb + 1) * 128],
                                qT[hp][0:64, ch * 512:(ch + 1) * 512])
                            nc.tensor.matmul(
                                supB[:, sl], kT[hp][64:128, jb * 128:(jb + 1) * 128],
                                qT[hp][64:128, ch * 512:(ch + 1) * 512])
                        return supA, supB

                    nxt = s_mm(0)
                    for s in range(ns):
                        supA, supB = nxt
                        ptA = ptp.tile([128, 1024], BF16, name="pt", tag="pt")
                        ptB = ptp.tile([128, 1024], BF16, name="pt", tag="pt")
                        nc.scalar.activation(ptA[:], supA[:],
                                             mybir.ActivationFunctionType.Exp)
                        nc.scalar.activation(ptB[:], supB[:],
                                             mybir.ActivationFunctionType.Exp)
                        if s + 1 < ns:
                            nxt = s_mm(s + 1)
                        if s >= ns - 2:  # diagonal supers: staircase mask
                            msl = slice(0, 1024) if s == ns - 2 else slice(1024, 2048)
                            nc.vector.tensor_mul(ptA[:], ptA[:], mask[:, msl])
                            nc.vector.tensor_mul(ptB[:], ptB[:], mask[:, msl])
                        for half, jb in ((0, 2 * s), (1, 2 * s + 1)):
                            sl = slice(512 * half, 512 * half + 512)
                            st_ = (s == 0 and half == 0)
                            sp_ = (s == ns - 1 and half == 1)
                            nc.tensor.matmul(
                                po0[0:65, :], vg[jb][:, hA * (DH + 1):(hA + 1) * (DH + 1)],
                                ptA[:, sl], start=st_, stop=sp_)
                            nc.tensor.matmul(
                                po1[0:65, :], vg[jb][:, hB * (DH + 1):(hB + 1) * (DH + 1)],
                                ptB[:, sl], start=st_, stop=sp_)

                    # D rows -> 32-aligned slots; O -> ot (unnormalized)
                    rA, rB = 32 * (hA % 4), 32 * (hB % 4)
                    nc.vector.tensor_copy(d4[hp][rA:rA + 1, :], po0[64:65, :])
                    nc.vector.tensor_copy(d4[hp][rB:rB + 1, :], po1[64:65, :])
                    nc.vector.tensor_copy(ot[ch][hp][0:64, :], po0[0:64, :])
                    nc.vector.tensor_copy(ot[ch][hp][64:128, :], po1[0:64, :])
                    if ch + 1 < NCH:
                        qproj_group(hp, ch + 1, dve_only=True)

                # ---- batched norm for this chunk ----
                for x in range(2):
                    dd = d4a if x == 0 else d4b
                    y0 = stp.tile([97, 512], F32, name="y0", tag="y0")
                    nc.vector.reciprocal_approx_fast(y0[:], dd[:])
                    rec = stp.tile([97, 512], BF16, name="rec", tag="rec")
                    nc.vector.tensor_copy(rec[:], y0[:])
                    for t in range(2):
                        ti = 2 * x + t
                        prep = ps.tile([128, 512], F32, name="pmix", tag="pmix")
                        nc.tensor.matmul(prep[:], esel[:, 128 * t:128 * (t + 1)],
                                         rec[:], start=True, stop=True)
                        nc.vector.tensor_mul(ot[ch][ti][:], ot[ch][ti][:], prep[:])

                # out-proj: filler for the next chunk's ACT-bound attention
                for db in range(DT):
                    outproj_group(ch, db)

            # phase 1 minimal prefix: everything attention ch0 needs
            for it in range(IT):
                kproj_group(it, 0)
            for r in range(4):
                vproj_group(r)
            for it in range(IT):
                qproj_group(it, 0)

            attention_chunk(0)

            # remaining projections: emitted after ch0 so they fill its
            # ACT-bound windows; all complete before the chunks that need them
            for rc in range(1, 4):
                for it in range(IT):
                    kproj_group(it, rc)
            for r in range(4, NB):
                vproj_group(r)

            for ch in range(1, NCH):
                attention_chunk(ch)

            p1cm.__exit__(None, None, None)

    nc.compile()
    return nc


def kernel(x, w_qkv, w_out, b_out):
    if "nc" not in _CACHE:
        _CACHE["nc"] = _build()
    nc = _CACHE["nc"]

    x = np.asarray(x, np.float32)
    w_qkv = np.asarray(w_qkv, np.float32)
    w_out = np.asarray(w_out, np.float32)
    b_out = np.asarray(b_out, np.float32)

    # staircase masks for the 4 diagonal block offsets:
    # mask_r[p, i] = 1 if p <= i - 128r ; layout [r0 | r1 | r2 | r3]
    p = np.arange(128)[:, None]
    i = np.arange(512)[None, :]
    msk2 = np.concatenate(
        [(p <= i - 128 * r).astype(np.float32) for r in range(4)], axis=1)

    # selector for denominator broadcast: [97, 256]
    ese = np.zeros((97, 256), np.float32)
    ese[0, 0:64] = 1.0      # even ti: head rows 0 (p<64), 32 (p>=64)
    ese[32, 64:128] = 1.0
    ese[64, 128:192] = 1.0  # odd ti: rows 64, 96
    ese[96, 192:256] = 1.0

    in_maps = []
    for c in range(8):
        b, g = c // 2, c % 2
        sl = slice(g * GI, (g + 1) * GI)
        in_maps.append(dict(
            xT=np.ascontiguousarray(x[b].T).astype(ml_dtypes.bfloat16),
            wq=(np.ascontiguousarray(w_qkv[:, sl]) * np.float32(SCALE)).astype(ml_dtypes.bfloat16),
            wk=np.ascontiguousarray(w_qkv[:, 1024 + g * GI:1024 + (g + 1) * GI]).astype(ml_dtypes.bfloat16),
            wv=np.ascontiguousarray(w_qkv[:, 2048 + g * GI:2048 + (g + 1) * GI]).astype(ml_dtypes.bfloat16),
            wo=np.ascontiguousarray(w_out[sl, :]).astype(ml_dtypes.bfloat16),
            msk=msk2.astype(ml_dtypes.bfloat16),
            ese=ese.astype(ml_dtypes.bfloat16),
            onv=np.ones((128, G), ml_dtypes.bfloat16),
        ))
    res = None
    for attempt in range(3):
        try:
            run_bass_kernel_spmd(nc, in_maps, core_ids=list(range(8)))  # warmup
            res = run_bass_kernel_spmd(nc, in_maps, core_ids=list(range(8)))
            break
        except Exception:
            if attempt == 2:
                raise
            time.sleep(10)
    _CACHE["res"] = res
    outs = [np.asarray(res.results[c]["out"], np.float32) for c in range(8)]
    full = np.empty((B, N, DIM), np.float32)
    for b in range(B):
        full[b] = (outs[2 * b] + outs[2 * b + 1]).T + b_out[None, :]
    return full
